# revision 1
# baseline (speedup 1.0000x reference)
# CATS-SwiGLU decode kernel for TRN2 (8 NeuronCores, SPMD tensor-parallel).
#
# Reference computation (decode path, B=S=1):
#   x1    = silu(x @ Wgatet)                  [1,1,dff]
#   flags = |x1| > threshold
#   z     = where(flags, (x @ Wup.T) * x1, 0) [1,1,dff]
#   out   = z @ Wdownt                        [1,1,d]
#
# Sharding: d_ff (11008) split across 8 cores (1376 rows each). Each core
# computes its z slice and a full-width partial down-projection; the host
# sums the 8 partials (the all-reduce of the TP hint, done on host).
#
# The gate/up GEMVs and most of the down GEMV run on the Vector engine as
# fused multiply+reduce (affine_mul_reduce) over weight tiles laid out
# rows-on-partitions (host-pretransposed where needed); DVE streams weights
# at ~444 GB/s, around per-core HBM rate, so the kernel is memory-bound at
# full fp32 precision.  z is replicated across partitions on the otherwise
# idle TensorEngine (transpose-matmul to a PSUM row, copy to SBUF,
# broadcast-matmul into PSUM).  The down-projection tail is split: d-chunks
# 0..19 reduce on DVE against the PSUM z_rep; d columns [2560, 4096) are
# computed on the TensorEngine (zm columns as stationary, natural-layout
# Wdownt as moving) so both engines drain the tail concurrently.
import sys

for _p in ("/opt/trn_rl_repo",):
    if _p not in sys.path:
        sys.path.insert(0, _p)

import numpy as np

import concourse.bass as bass
import concourse.tile as tile
from concourse import bacc, mybir
from concourse.bass_utils import run_bass_kernel_spmd
from concourse.masks import make_identity

D = 4096
FF = 11008
NCORES = 8
FSH = FF // NCORES          # 1376 rows of d_ff per core
NCH = (FSH + 127) // 128    # 11 chunks of <=128 rows
LAST = FSH - 128 * (NCH - 1)  # 96 rows in the last chunk
NDG = 16                    # down-proj groups: 2 d-chunks (256 d) each
DPE0 = 2 * NDG * 128        # 2560: first d column of the PE share
DPE = D - DPE0              # 1536 PE-share columns (= 3 x 512)
CSPLIT = 9                  # z batch 1 = chunks [0, 9); batch 2 stays tiny
F32 = mybir.dt.float32

_CACHE = {}


def _bcast(ap, parts):
    """Replicate a 1-D AP across `parts` partitions (0-stride partition dim)."""
    return bass.AP(tensor=ap.tensor, offset=ap.offset, ap=[[0, parts]] + list(ap.ap))


def _build_nc():
    nc = bacc.Bacc("TRN2", target_bir_lowering=False, debug=False)

    x_d = nc.dram_tensor("x", [D], F32, kind="ExternalInput")
    wg_d = nc.dram_tensor("wg", [FSH, D], F32, kind="ExternalInput")
    wu_d = nc.dram_tensor("wu", [FSH, D], F32, kind="ExternalInput")
    wd_d = nc.dram_tensor("wd", [NDG, 128, 2 * FSH], F32, kind="ExternalInput")
    thr_d = nc.dram_tensor("thr", [1], F32, kind="ExternalInput")
    out_d = nc.dram_tensor("out", [128, 2 * NDG], F32, kind="ExternalOutput")

    with tile.TileContext(nc) as tc:
        with (
            tc.tile_pool(name="const", bufs=1) as const_pool,
            tc.tile_pool(name="wpool", bufs=4) as wpool,
            tc.tile_pool(name="apool", bufs=4) as apool,
            tc.tile_pool(name="acts", bufs=1) as acts,
            tc.tile_pool(name="psum", bufs=1, space="PSUM") as psum,
        ):
            # constants on the scalar (qAct) ring so the weight stream on
            # the sync (qSP) ring starts at t=0
            x_rep = const_pool.tile([128, D], F32)
            nc.scalar.dma_start(out=x_rep[:], in_=_bcast(x_d.ap(), 128))
            thr_sb = const_pool.tile([128, 1], F32)
            nc.scalar.dma_start(out=thr_sb[:], in_=_bcast(thr_d.ap(), 128))

            x1 = acts.tile([128, NCH], F32)  # gate pre-activation
            u = acts.tile([128, NCH], F32)   # up projection
            zm = acts.tile([128, NCH], F32)  # masked z
            nc.vector.memset(x1[:], 0.0)
            nc.vector.memset(u[:], 0.0)

            # warm the sigmoid/abs ACT tables while the DMA stream runs
            warm = acts.tile([128, 1], F32)
            nc.scalar.activation(
                warm[:], thr_sb[:], mybir.ActivationFunctionType.Sigmoid
            )
            nc.scalar.activation(
                warm[:], thr_sb[:], mybir.ActivationFunctionType.Abs
            )

            # z replication machinery (TensorEngine)
            ident = const_pool.tile([128, 128], F32)
            make_identity(nc, ident[:])
            ones_row = const_pool.tile([1, 128], F32)
            nc.vector.memset(ones_row[:], 1.0)
            z_row_ps = psum.tile([1, NCH * 128], F32)
            z_row_sb = const_pool.tile([1, NCH * 128], F32)
            z_rep = psum.tile([128, NCH * 128], F32)
            batches = ((0, CSPLIT), (CSPLIT, NCH))

            def z_batch_compute(bi):
                c0, c1 = batches[bi]
                cs = slice(c0, c1)
                sg = acts.tile([128, NCH], F32, tag="sg", name="sg")
                nc.scalar.activation(
                    sg[:, cs], x1[:, cs], mybir.ActivationFunctionType.Sigmoid
                )
                x1s = acts.tile([128, NCH], F32, tag="x1s", name="x1s")
                nc.vector.tensor_mul(x1s[:, cs], x1[:, cs], sg[:, cs])
                absx = acts.tile([128, NCH], F32, tag="absx", name="absx")
                nc.scalar.activation(
                    absx[:, cs], x1s[:, cs], mybir.ActivationFunctionType.Abs
                )
                mask = acts.tile([128, NCH], F32, tag="mask", name="mask")
                nc.vector.tensor_scalar(
                    out=mask[:, cs],
                    in0=absx[:, cs],
                    scalar1=thr_sb[:],
                    scalar2=None,
                    op0=mybir.AluOpType.is_gt,
                )
                z = acts.tile([128, NCH], F32, tag="z", name="z")
                nc.vector.tensor_mul(z[:, cs], u[:, cs], x1s[:, cs])
                nc.vector.tensor_mul(zm[:, cs], z[:, cs], mask[:, cs])

            def z_batch_rep(bi):
                c0, c1 = batches[bi]
                for c in range(c0, c1):
                    pc = 128 if c < NCH - 1 else LAST
                    fs = slice(c * 128, c * 128 + pc)
                    nc.tensor.matmul(
                        out=z_row_ps[0:1, fs],
                        lhsT=zm[:pc, c : c + 1],
                        rhs=ident[:pc, :pc],
                        start=True,
                        stop=True,
                    )
                    nc.scalar.copy(z_row_sb[0:1, fs], z_row_ps[0:1, fs])
                    nc.tensor.matmul(
                        out=z_rep[:, fs],
                        lhsT=ones_row[0:1, :],
                        rhs=z_row_sb[0:1, fs],
                        start=True,
                        stop=True,
                    )

            # gate and up GEMVs: acc[p, c] = sum_j W[c*128+p, j] * x[j].
            # All weight DMAs stay back-to-back on the sync ring; batch-1 z
            # compute + PE replication are emitted mid-up-loop so they
            # overlap the stream.
            for wi, (wdram, acc) in enumerate(((wg_d, x1), (wu_d, u))):
                for c in range(NCH):
                    p = 128 if c < NCH - 1 else LAST
                    wt = wpool.tile([128, D], F32, tag="w", name="wt")
                    nc.sync.dma_start(
                        out=wt[:p, :], in_=wdram.ap()[c * 128 : c * 128 + p, :]
                    )
                    nc.vector.affine_mul_reduce(
                        out=wt[:p, :],
                        accum_out=acc[:p, c : c + 1],
                        in0=wt[:p, :],
                        in1=x_rep[:p, :],
                        scale=1.0,
                        bias=0.0,
                    )
                    if wi == 1 and c == CSPLIT - 1:
                        z_batch_compute(0)
                        z_batch_rep(0)
            z_batch_compute(1)
            z_batch_rep(1)

            # down projection: osb[p, c] = sum_f WdT[c*128+p, f] * z[f]
            osb = acts.tile([128, 2 * NDG], F32)
            for g in range(NDG):
                dt_ = apool.tile([128, 2 * FSH], F32, tag="wd", name="dt_")
                nc.scalar.dma_start(out=dt_[:], in_=wd_d.ap()[g])
                for h in range(2):
                    sl = slice(h * FSH, (h + 1) * FSH)
                    nc.vector.affine_mul_reduce(
                        out=dt_[:, sl],
                        accum_out=osb[:, 2 * g + h : 2 * g + h + 1],
                        in0=dt_[:, sl],
                        in1=z_rep[:, 0:FSH],
                        scale=1.0,
                        bias=0.0,
                    )

            nc.sync.dma_start(out=out_d.ap(), in_=osb[:])

    nc.compile()
    return nc


def _get_nc():
    if "nc" not in _CACHE:
        _CACHE["nc"] = _build_nc()
    return _CACHE["nc"]


def make_in_maps(x, Wup, Wgatet, Wdownt, threshold):
    """Shard full inputs into the 8 per-core input maps."""
    x_flat = np.ascontiguousarray(np.asarray(x, dtype=np.float32).reshape(D))
    thr = np.asarray(threshold, dtype=np.float32).reshape(1)
    Wup = np.asarray(Wup, dtype=np.float32)
    Wgatet = np.asarray(Wgatet, dtype=np.float32)
    Wdownt = np.asarray(Wdownt, dtype=np.float32)
    in_maps = []
    for i in range(NCORES):
        sl = slice(i * FSH, (i + 1) * FSH)
        wg = np.ascontiguousarray(Wgatet[:, sl].T)          # [FSH, D]
        wu = np.ascontiguousarray(Wup[sl, :])               # [FSH, D]
        wdt = np.ascontiguousarray(Wdownt[sl, :].T)         # [D, FSH]
        a = wdt.reshape(2 * NDG, 128, FSH)
        wd = np.ascontiguousarray(
            np.concatenate([a[0::2], a[1::2]], axis=2)
        )                                                   # [NDG, 128, 2*FSH]
        in_maps.append({"x": x_flat, "wg": wg, "wu": wu, "wd": wd, "thr": thr})
    return in_maps


def run_sharded(x, Wup, Wgatet, Wdownt, threshold, trace=False, tmpdir=None):
    """Run on the 8 NeuronCores; returns (full_output, BassKernelResults)."""
    nc = _get_nc()
    in_maps = make_in_maps(x, Wup, Wgatet, Wdownt, threshold)
    res = run_bass_kernel_spmd(
        nc, in_maps, list(range(NCORES)), trace=trace, tmpdir=tmpdir
    )
    # un-shard: osb[p, c] holds partial_out[c*128 + p]; sum partials over cores
    acc = np.zeros(D, dtype=np.float64)
    for r in res.results:
        acc += r["out"].T.reshape(D).astype(np.float64)
    out = acc.astype(np.float32).reshape(1, 1, D)
    return out, res


def kernel(x, Wup, Wgatet, Wdownt, threshold):
    out, _ = run_sharded(x, Wup, Wgatet, Wdownt, threshold)
    return out



# revision 6
# speedup vs baseline: 1.4529x; 1.4529x over previous
# CATS-SwiGLU decode kernel for TRN2 (8 NeuronCores, SPMD tensor-parallel).
#
# Reference computation (decode path, B=S=1):
#   x1    = silu(x @ Wgatet)                  [1,1,dff]
#   flags = |x1| > threshold
#   z     = where(flags, (x @ Wup.T) * x1, 0) [1,1,dff]
#   out   = z @ Wdownt                        [1,1,d]
#
# Sharding: d_ff (11008) split across 8 cores (1376 rows each). Each core
# computes its z slice and a full-width partial down-projection; the host
# sums the 8 partials (the all-reduce of the TP hint, done on host).
#
# The kernel is memory-bound: all three weight matrices are converted to
# bf16 on the host (halving HBM traffic vs fp32; quantization error
# ~0.1% RMS, far under the 2e-2 gate) and streamed through the
# TensorEngine as the *moving* operand of accumulating GEMV matmuls
# (bf16 moving = 1 row/cycle = ~614 GB/s consumption, above the ~430
# GB/s per-core DMA fabric rate, so DMA stays the bottleneck).  Weights
# are laid out per-partition-contiguous in DRAM so every DMA descriptor
# moves 22-24 KB contiguous runs per partition, and the stream is split
# across two HWDGE queues (sync + gpsimd) to hide descriptor handoff.
# Gate/up GEMVs accumulate into [1, d_ff] PSUM rows; the tiny silu/flag
# elementwise chain runs on ACT/DVE on partition 0 in the shadow of the
# up-weight stream; z is transposed to chunk-column layout [128, 11] with
# K=1 matmuls so the down GEMV can use it as the stationary operand.
import sys

for _p in ("/opt/trn_rl_repo",):
    if _p not in sys.path:
        sys.path.insert(0, _p)

import numpy as np
import ml_dtypes

import concourse.bass as bass
import concourse.tile as tile
from concourse import bacc, mybir
from concourse.bass_utils import run_bass_kernel_spmd

D = 4096
FF = 11008
NCORES = 8
FSH = FF // NCORES            # 1376 rows of d_ff per core
NCD = D // 128                # 32 contraction chunks for gate/up
NCF = (FSH + 127) // 128      # 11 contraction chunks for down
LASTF = FSH - 128 * (NCF - 1)  # 96 rows in the last f chunk
FT = ((0, 512), (512, 512), (1024, 352))  # gate/up psum f-tiles (bank-sized)
NQ = 8                        # wg/wu stream pieces per matrix (4 chunks each)
CPQ = NCD // NQ               # contraction chunks per piece
QW = CPQ * FSH                # piece width in elements per partition
WDQ = (3, 3, 3, 2)            # wd stream pieces, in f-chunks
F32 = mybir.dt.float32
BF16 = mybir.dt.bfloat16
BF = ml_dtypes.bfloat16

_CACHE = {}


def _build_nc():
    nc = bacc.Bacc("TRN2", target_bir_lowering=False, debug=False)

    x_d = nc.dram_tensor("x", [128, NCD], BF16, kind="ExternalInput")
    wg_d = nc.dram_tensor("wg", [128, NCD * FSH], BF16, kind="ExternalInput")
    wu_d = nc.dram_tensor("wu", [128, NCD * FSH], BF16, kind="ExternalInput")
    wd_d = nc.dram_tensor("wd", [128, NCF * D], BF16, kind="ExternalInput")
    thr_d = nc.dram_tensor("thr", [1], F32, kind="ExternalInput")
    out_d = nc.dram_tensor("out", [D], F32, kind="ExternalOutput")

    with tile.TileContext(nc) as tc:
        with (
            tc.tile_pool(name="const", bufs=1) as const_pool,
            tc.tile_pool(name="wpool", bufs=3) as wpool,
            tc.tile_pool(name="acts", bufs=1) as acts,
        ):
            # small constants on the gpsimd ring ahead of its weight pieces
            xcol = const_pool.tile([128, NCD], BF16)
            nc.gpsimd.dma_start(out=xcol[:], in_=x_d.ap())
            thr_sb = const_pool.tile([1, 1], F32)
            nc.gpsimd.dma_start(out=thr_sb[:], in_=thr_d.ap())
            ones_bf = const_pool.tile([1, 1], BF16)
            nc.vector.memset(ones_bf[:], 1.0)

            # activation scratch (partition 0)
            sg = acts.tile([1, FSH], F32)
            x1s = acts.tile([1, FSH], F32)
            ab = acts.tile([1, FSH], F32)
            mk = acts.tile([1, FSH], F32)
            xm = acts.tile([1, FSH], F32)
            z_row = acts.tile([1, NCF * 128], BF16)
            nc.vector.memset(z_row[:], 0.0)
            z_bf = acts.tile([128, NCF], BF16)
            nc.vector.memset(z_bf[:], 0.0)
            osb = acts.tile([1, D], F32)
            # warm the ACT tables used later while the DMAs stream
            warm = acts.tile([1, 1], F32)
            nc.scalar.activation(
                warm[:], thr_sb[:], mybir.ActivationFunctionType.Sigmoid
            )
            nc.scalar.activation(
                warm[:], thr_sb[:], mybir.ActivationFunctionType.Abs
            )
            nc.scalar.copy(warm[:], thr_sb[:])

            # whole down matrix lands in SBUF while gate/up stream runs
            wd_sb = acts.tile([128, NCF * D], BF16)

            qs = (nc.sync, nc.gpsimd)
            wtiles = {}

            def emit_w_dma(i):
                wdram = wg_d if i < NQ else wu_d
                q = i % NQ
                t = wpool.tile([128, QW], BF16, tag="w", name="wt")
                qs[i % 2].dma_start(
                    out=t[:], in_=wdram.ap()[:, q * QW : (q + 1) * QW]
                )
                wtiles[i] = t

            def emit_wd_dmas():
                off = 0
                for p, nchunks in enumerate(WDQ):
                    w = nchunks * D
                    qs[p % 2].dma_start(
                        out=wd_sb[:, off : off + w],
                        in_=wd_d.ap()[:, off : off + w],
                    )
                    off += w

            with tc.tile_pool(name="ps1", bufs=1, space="PSUM") as ps1:
                pg = ps1.tile([1, 1536], F32)
                pu = ps1.tile([1, 1536], F32)
                pz = ps1.tile([128, NCF], F32)

                emit_w_dma(0)
                emit_w_dma(1)
                emit_w_dma(2)
                for i in range(2 * NQ):
                    accp = pg if i < NQ else pu
                    wt = wtiles.pop(i)
                    for cc in range(CPQ):
                        c = (i % NQ) * CPQ + cc
                        for toff, tlen in FT:
                            nc.tensor.matmul(
                                out=accp[0:1, toff : toff + tlen],
                                lhsT=xcol[:, c : c + 1],
                                rhs=wt[:, cc * FSH + toff : cc * FSH + toff + tlen],
                                start=(c == 0),
                                stop=(c == NCD - 1),
                            )
                    if i + 3 < 2 * NQ:
                        emit_w_dma(i + 3)
                    if i == 2 * NQ - 4:
                        # queue the down matrix behind the last up piece
                        emit_wd_dmas()
                    if i == NQ - 1:
                        # gate done: silu + threshold mask on partition 0,
                        # hidden under the up-weight stream
                        nc.scalar.activation(
                            sg[:], pg[0:1, 0:FSH],
                            mybir.ActivationFunctionType.Sigmoid,
                        )
                        nc.vector.tensor_mul(x1s[:], pg[0:1, 0:FSH], sg[:])
                        nc.scalar.activation(
                            ab[:], x1s[:], mybir.ActivationFunctionType.Abs
                        )
                        nc.vector.tensor_scalar(
                            out=mk[:],
                            in0=ab[:],
                            scalar1=thr_sb[:],
                            scalar2=None,
                            op0=mybir.AluOpType.is_gt,
                        )
                        nc.vector.tensor_mul(xm[:], x1s[:], mk[:])

                # z = up * silu(gate) * flags, as bf16 on partition 0
                nc.vector.tensor_mul(z_row[0:1, 0:FSH], pu[0:1, 0:FSH], xm[:])
                # transpose z to chunk-column layout [128, NCF] for the
                # down GEMV's stationary operand (K=1 matmul per chunk)
                for c in range(NCF):
                    pc = 128 if c < NCF - 1 else LASTF
                    nc.tensor.matmul(
                        out=pz[0:pc, c : c + 1],
                        lhsT=z_row[0:1, c * 128 : c * 128 + pc],
                        rhs=ones_bf[:],
                        start=True,
                        stop=True,
                    )
                    nc.scalar.copy(z_bf[0:pc, c : c + 1], pz[0:pc, c : c + 1])

            with tc.tile_pool(name="ps2", bufs=1, space="PSUM") as ps2:
                pd = ps2.tile([1, D], F32)
                for c in range(NCF):
                    pc = 128 if c < NCF - 1 else LASTF
                    for dt in range(8):
                        nc.tensor.matmul(
                            out=pd[0:1, dt * 512 : (dt + 1) * 512],
                            lhsT=z_bf[0:pc, c : c + 1],
                            rhs=wd_sb[0:pc, c * D + dt * 512 : c * D + (dt + 1) * 512],
                            start=(c == 0),
                            stop=(c == NCF - 1),
                        )
                # drain the 8 psum banks to SBUF on two engines in parallel
                # (gpsimd cannot access PSUM)
                for dt in range(8):
                    sl = slice(dt * 512, (dt + 1) * 512)
                    if dt % 2 == 0:
                        nc.scalar.copy(osb[0:1, sl], pd[0:1, sl])
                    else:
                        nc.vector.tensor_scalar_add(osb[0:1, sl], pd[0:1, sl], 0.0)

            nc.sync.dma_start(out=out_d.ap(), in_=osb[:])

    nc.compile()
    return nc


def _get_nc():
    if "nc" not in _CACHE:
        _CACHE["nc"] = _build_nc()
    return _CACHE["nc"]


def make_in_maps(x, Wup, Wgatet, Wdownt, threshold):
    """Shard full inputs into the 8 per-core input maps (bf16 weights)."""
    x_flat = np.asarray(x, dtype=np.float32).reshape(D)
    xcol = np.ascontiguousarray(x_flat.reshape(NCD, 128).T).astype(BF)
    thr = np.asarray(threshold, dtype=np.float32).reshape(1)
    Wup = np.asarray(Wup, dtype=np.float32)
    Wgatet = np.asarray(Wgatet, dtype=np.float32)
    Wdownt = np.asarray(Wdownt, dtype=np.float32)
    in_maps = []
    for i in range(NCORES):
        sl = slice(i * FSH, (i + 1) * FSH)
        # gate: [d, f] chunk-major -> [128, NCD*FSH] per-partition contiguous
        wg = (
            Wgatet[:, sl]
            .reshape(NCD, 128, FSH)
            .transpose(1, 0, 2)
            .reshape(128, NCD * FSH)
            .astype(BF)
        )
        wu = (
            Wup[sl, :]
            .T.reshape(NCD, 128, FSH)
            .transpose(1, 0, 2)
            .reshape(128, NCD * FSH)
            .astype(BF)
        )
        wd_pad = np.zeros((NCF * 128, D), dtype=np.float32)
        wd_pad[:FSH] = Wdownt[sl, :]
        wd = (
            wd_pad.reshape(NCF, 128, D)
            .transpose(1, 0, 2)
            .reshape(128, NCF * D)
            .astype(BF)
        )
        in_maps.append(
            {
                "x": xcol,
                "wg": np.ascontiguousarray(wg),
                "wu": np.ascontiguousarray(wu),
                "wd": np.ascontiguousarray(wd),
                "thr": thr,
            }
        )
    return in_maps


def run_sharded(x, Wup, Wgatet, Wdownt, threshold, trace=False, tmpdir=None):
    """Run on the 8 NeuronCores; returns (full_output, BassKernelResults)."""
    nc = _get_nc()
    in_maps = make_in_maps(x, Wup, Wgatet, Wdownt, threshold)
    res = run_bass_kernel_spmd(
        nc, in_maps, list(range(NCORES)), trace=trace, tmpdir=tmpdir
    )
    acc = np.zeros(D, dtype=np.float64)
    for r in res.results:
        acc += r["out"].reshape(D).astype(np.float64)
    out = acc.astype(np.float32).reshape(1, 1, D)
    return out, res


def kernel(x, Wup, Wgatet, Wdownt, threshold):
    out, _ = run_sharded(x, Wup, Wgatet, Wdownt, threshold)
    return out


# revision 9
# speedup vs baseline: 1.6158x; 1.1122x over previous
# CATS-SwiGLU decode kernel for TRN2 (8 NeuronCores, SPMD tensor-parallel).
# v3: bf16 weights streamed to BOTH the TensorEngine and the Vector engine
# concurrently (each sustains ~125-250 Gelem/s; combined they outrun the
# ~430 GB/s per-core DMA fabric), with the down-projection weights pinned
# behind the gate/up stream via an artificial dependency so the Tile list
# scheduler cannot hoist them ahead.
#
#   x1    = silu(x @ Wgatet); flags = |x1| > thr
#   z     = where(flags, (x @ Wup.T) * x1, 0);  out = z @ Wdownt
#
# f-split for gate/up: f-rows [0:FD) are computed by DVE affine_mul_reduce
# over f-major tiles [128f, 4096d] (accumulator lands f-on-partitions, i.e.
# already transposed for the down GEMV); f-rows [FD:1376) go through the PE
# as accumulating GEMV matmuls (d-chunk stationary x column, weights
# moving).  d-split for down: d-cols [0:DP) on PE (f-major weights,
# z-column stationary), d-cols [DP:4096) on DVE (d-major weights times a
# z broadcast built with a ones-column PE matmul).
import sys

for _p in ("/opt/trn_rl_repo",):
    if _p not in sys.path:
        sys.path.insert(0, _p)

import numpy as np
import ml_dtypes

import concourse.bass as bass
import concourse.tile as tile
from concourse import bacc, mybir
from concourse.bass_utils import run_bass_kernel_spmd
from concourse.masks import make_identity

D = 4096
FF = 11008
NCORES = 8
FSH = FF // NCORES            # 1376 rows of d_ff per core
NCD = D // 128                # 32 contraction chunks for gate/up PE part
NCF = (FSH + 127) // 128      # 11 f chunks
LASTF = FSH - 128 * (NCF - 1)  # 96 rows in the last f chunk

FDC = 6                       # f chunks on DVE for gate/up
FD = FDC * 128                # 768
FP = FSH - FD                 # 608 PE-part f width
FTP = ((0, 512), (512, FP - 512))   # PE-part psum f-tiles
NPP = 4                       # PE-part stream pieces per matrix (8 chunks)
CPP = NCD // NPP              # 8 chunks per PE piece
PPW = CPP * FP                # PE piece width (elems/partition)

DPC = 16                      # d-cols on PE for down, in units of 128
DP = DPC * 128                # 2048
NDVG = (D - DP) // 128        # 16 DVE down groups

F32 = mybir.dt.float32
BF16 = mybir.dt.bfloat16
BF = ml_dtypes.bfloat16

_CACHE = {}


def _bcast(ap, parts):
    return bass.AP(tensor=ap.tensor, offset=ap.offset, ap=[[0, parts]] + list(ap.ap))


def _build_nc():
    nc = bacc.Bacc("TRN2", target_bir_lowering=False, debug=False)

    xc_d = nc.dram_tensor("xc", [128, NCD], BF16, kind="ExternalInput")
    xf_d = nc.dram_tensor("xf", [D], BF16, kind="ExternalInput")
    wgd_d = nc.dram_tensor("wgd", [128, FDC * D], BF16, kind="ExternalInput")
    wud_d = nc.dram_tensor("wud", [128, FDC * D], BF16, kind="ExternalInput")
    wgp_d = nc.dram_tensor("wgp", [128, NCD * FP], BF16, kind="ExternalInput")
    wup_d = nc.dram_tensor("wup", [128, NCD * FP], BF16, kind="ExternalInput")
    wdp_d = nc.dram_tensor("wdp", [128, NCF * DP], BF16, kind="ExternalInput")
    wdd_d = nc.dram_tensor("wdd", [128, NDVG * FSH], BF16, kind="ExternalInput")
    thr_d = nc.dram_tensor("thr", [1], F32, kind="ExternalInput")
    outp_d = nc.dram_tensor("outp", [DP], F32, kind="ExternalOutput")
    outd_d = nc.dram_tensor("outd", [128, NDVG], F32, kind="ExternalOutput")

    with tile.TileContext(nc) as tc:
        with (
            tc.tile_pool(name="const", bufs=1) as cp,
            tc.tile_pool(name="ppool", bufs=3) as ppool,
            tc.tile_pool(name="tpool", bufs=3) as tpool,
            tc.tile_pool(name="acts", bufs=1) as acts,
        ):
            # constants ride the gpsimd ring ahead of its weight pieces
            xcol = cp.tile([128, NCD], BF16)
            nc.gpsimd.dma_start(out=xcol[:], in_=xc_d.ap())
            xrep = cp.tile([128, D], BF16)
            nc.gpsimd.dma_start(out=xrep[:], in_=_bcast(xf_d.ap(), 128))
            thr_sb = cp.tile([128, 1], F32)
            nc.gpsimd.dma_start(out=thr_sb[:], in_=_bcast(thr_d.ap(), 128))
            ones_col = cp.tile([1, 128], BF16)
            nc.vector.memset(ones_col[:], 1.0)
            ident = cp.tile([128, 128], BF16)
            make_identity(nc, ident[:])

            # activation scratch
            accg = acts.tile([128, FDC], F32)   # DVE-part gate accum
            accu = acts.tile([128, FDC], F32)   # DVE-part up accum
            sgd = acts.tile([128, FDC], F32)
            x1d = acts.tile([128, FDC], F32)
            abd = acts.tile([128, FDC], F32)
            mkd = acts.tile([128, FDC], F32)
            xmd = acts.tile([128, FDC], F32)
            sgp = acts.tile([1, FP], F32)
            x1p = acts.tile([1, FP], F32)
            abp = acts.tile([1, FP], F32)
            mkp = acts.tile([1, FP], F32)
            xmp = acts.tile([1, FP], F32)
            z_row = acts.tile([1, NCF * 128], BF16)
            nc.vector.memset(z_row[:], 0.0)
            z_bf = acts.tile([128, NCF], BF16)
            nc.vector.memset(z_bf[:], 0.0)
            zrep = acts.tile([128, FSH], BF16)
            osbp = acts.tile([1, DP], F32)
            outd_sb = acts.tile([128, NDVG], F32)
            warm = acts.tile([128, 1], F32)
            nc.scalar.activation(
                warm[:], thr_sb[:], mybir.ActivationFunctionType.Sigmoid
            )
            nc.scalar.activation(
                warm[:], thr_sb[:], mybir.ActivationFunctionType.Abs
            )
            nc.scalar.copy(warm[:], thr_sb[:])

            # resident down-weight tiles (DMAs gated behind the gate/up
            # stream by a dummy dependency, emitted later)
            wdp_sb = acts.tile([128, NCF * DP], BF16)
            wdd_sb = acts.tile([128, NDVG * FSH], BF16)

            qs = (nc.sync, nc.gpsimd)
            nq = {"i": 0}

            def next_q():
                q = qs[nq["i"] % 2]
                nq["i"] += 1
                return q

            # ---- gate/up weight stream: interleave PE pieces and DVE tiles
            # per matrix in arrival-proportional order ----
            # per-matrix descriptor order: P0 T0 T1 P1 T2 T3 P2 T4 T5 P3
            ORDER = ("P", "T", "T", "P", "T", "T", "P", "T", "T", "P")
            ptiles = {}
            dtiles = {}

            def emit_piece(mi, kind, idx):
                if kind == "P":
                    dram = wgp_d if mi == 0 else wup_d
                    t = ppool.tile([128, PPW], BF16, tag="pw", name="pw")
                    next_q().dma_start(
                        out=t[:], in_=dram.ap()[:, idx * PPW : (idx + 1) * PPW]
                    )
                    ptiles[(mi, idx)] = t
                else:
                    dram = wgd_d if mi == 0 else wud_d
                    t = tpool.tile([128, D], BF16, tag="tw", name="tw")
                    next_q().dma_start(
                        out=t[:], in_=dram.ap()[:, idx * D : (idx + 1) * D]
                    )
                    dtiles[(mi, idx)] = t

            # emission list: (mi, kind, idx) in stream order
            stream = []
            for mi in range(2):
                cnt = {"P": 0, "T": 0}
                for kind in ORDER:
                    stream.append((mi, kind, cnt[kind]))
                    cnt[kind] += 1

            with tc.tile_pool(name="ps1", bufs=1, space="PSUM") as ps1:
                pgp = ps1.tile([1, 1024], F32)
                pup = ps1.tile([1, 1024], F32)
                pzrow = ps1.tile([1, FDC * 128], F32)
                pzcol = ps1.tile([128, NCF - FDC], F32)

                # prefetch depth 3 on each stream type
                emitted = 0
                for k in range(3):
                    emit_piece(*stream[k])
                    emitted += 1

                def consume(mi, kind, idx):
                    if kind == "P":
                        accp = pgp if mi == 0 else pup
                        t = ptiles[(mi, idx)]
                        for cc in range(CPP):
                            c = idx * CPP + cc
                            for toff, tlen in FTP:
                                nc.tensor.matmul(
                                    out=accp[0:1, toff : toff + tlen],
                                    lhsT=xcol[:, c : c + 1],
                                    rhs=t[:, cc * FP + toff : cc * FP + toff + tlen],
                                    start=(c == 0),
                                    stop=(c == NCD - 1),
                                )
                    else:
                        acct = accg if mi == 0 else accu
                        t = dtiles[(mi, idx)]
                        nc.vector.affine_mul_reduce(
                            out=t[:],
                            accum_out=acct[:, idx : idx + 1],
                            in0=t[:],
                            in1=xrep[:],
                            scale=1.0,
                            bias=0.0,
                        )

                wd_gated = {"done": False}

                def emit_wd_dmas():
                    # dummy write into each destination tile, reading the
                    # last gate-stream tile: pins the wd DMAs (WAW dep)
                    # until ~half the gate/up bytes have arrived.
                    trigger = ptiles[(0, NPP - 1)]
                    nc.gpsimd.tensor_scalar_add(
                        wdp_sb[0:1, 0:1], trigger[0:1, 0:1], 0.0
                    )
                    nc.gpsimd.tensor_scalar_add(
                        wdd_sb[0:1, 0:1], trigger[0:1, 0:1], 0.0
                    )
                    # interleave PE-part and DVE-part pieces on the gpsimd
                    # queue (the sync queue keeps draining gate/up)
                    pbounds = [0, 3, 6, 9, NCF]
                    dbounds = [0, 4, 8, 12, NDVG]
                    for k in range(4):
                        p0, p1 = pbounds[k], pbounds[k + 1]
                        nc.gpsimd.dma_start(
                            out=wdp_sb[:, p0 * DP : p1 * DP],
                            in_=wdp_d.ap()[:, p0 * DP : p1 * DP],
                        )
                        g0, g1 = dbounds[k], dbounds[k + 1]
                        nc.gpsimd.dma_start(
                            out=wdd_sb[:, g0 * FSH : g1 * FSH],
                            in_=wdd_d.ap()[:, g0 * FSH : g1 * FSH],
                        )

                for k, item in enumerate(stream):
                    consume(*item)
                    if emitted < len(stream):
                        emit_piece(*stream[emitted])
                        emitted += 1
                    if emitted == len(stream) and not wd_gated["done"]:
                        wd_gated["done"] = True
                        emit_wd_dmas()

                # ---- gate elementwise ----
                # DVE part [128, FDC]
                nc.scalar.activation(
                    sgd[:], accg[:], mybir.ActivationFunctionType.Sigmoid
                )
                nc.vector.tensor_mul(x1d[:], accg[:], sgd[:])
                nc.scalar.activation(
                    abd[:], x1d[:], mybir.ActivationFunctionType.Abs
                )
                nc.vector.tensor_scalar(
                    out=mkd[:], in0=abd[:], scalar1=thr_sb[:], scalar2=None,
                    op0=mybir.AluOpType.is_gt,
                )
                nc.vector.tensor_mul(xmd[:], x1d[:], mkd[:])
                # PE part [1, FP]
                nc.scalar.activation(
                    sgp[:], pgp[0:1, 0:FP], mybir.ActivationFunctionType.Sigmoid
                )
                nc.vector.tensor_mul(x1p[:], pgp[0:1, 0:FP], sgp[:])
                nc.scalar.activation(
                    abp[:], x1p[:], mybir.ActivationFunctionType.Abs
                )
                nc.vector.tensor_scalar(
                    out=mkp[:], in0=abp[:], scalar1=thr_sb[0:1, :], scalar2=None,
                    op0=mybir.AluOpType.is_gt,
                )
                nc.vector.tensor_mul(xmp[:], x1p[:], mkp[:])

                # ---- z ----
                # DVE part -> z_bf columns [0:FDC) directly (bf16 convert)
                nc.vector.tensor_mul(z_bf[:, 0:FDC], accu[:], xmd[:])
                # PE part -> z_row slice [FD:FSH)
                nc.vector.tensor_mul(
                    z_row[0:1, FD:FSH], pup[0:1, 0:FP], xmp[:]
                )
                # DVE-part z to row form (for the z broadcast): col -> row
                for c in range(FDC):
                    nc.tensor.matmul(
                        out=pzrow[0:1, c * 128 : (c + 1) * 128],
                        lhsT=z_bf[:, c : c + 1],
                        rhs=ident[:],
                        start=True,
                        stop=True,
                    )
                    nc.scalar.copy(
                        z_row[0:1, c * 128 : (c + 1) * 128],
                        pzrow[0:1, c * 128 : (c + 1) * 128],
                    )
                # PE-part z to column form: z_bf columns [FDC:NCF)
                for c in range(FDC, NCF):
                    pc = 128 if c < NCF - 1 else LASTF
                    nc.tensor.matmul(
                        out=pzcol[0:pc, c - FDC : c - FDC + 1],
                        lhsT=z_row[0:1, c * 128 : c * 128 + pc],
                        rhs=ones_col[0:1, 0:1],
                        start=True,
                        stop=True,
                    )
                    nc.scalar.copy(
                        z_bf[0:pc, c : c + 1], pzcol[0:pc, c - FDC : c - FDC + 1]
                    )

            with tc.tile_pool(name="ps2", bufs=1, space="PSUM") as ps2:
                pdp = ps2.tile([1, DP], F32)
                przep = ps2.tile([128, 1536], F32)
                # z broadcast [128, FSH] for the DVE down part
                for toff, tlen in ((0, 512), (512, 512), (1024, FSH - 1024)):
                    nc.tensor.matmul(
                        out=przep[:, toff : toff + tlen],
                        lhsT=ones_col[:],
                        rhs=z_row[0:1, toff : toff + tlen],
                        start=True,
                        stop=True,
                    )
                nc.scalar.copy(zrep[:], przep[:, 0:FSH])

                # PE down part: d-cols [0:DP)
                for c in range(NCF):
                    pc = 128 if c < NCF - 1 else LASTF
                    for dt in range(DP // 512):
                        nc.tensor.matmul(
                            out=pdp[0:1, dt * 512 : (dt + 1) * 512],
                            lhsT=z_bf[0:pc, c : c + 1],
                            rhs=wdp_sb[0:pc, c * DP + dt * 512 : c * DP + (dt + 1) * 512],
                            start=(c == 0),
                            stop=(c == NCF - 1),
                        )
                # DVE down part: d-cols [DP:D)
                for g in range(NDVG):
                    nc.vector.affine_mul_reduce(
                        out=wdd_sb[:, g * FSH : (g + 1) * FSH],
                        accum_out=outd_sb[:, g : g + 1],
                        in0=wdd_sb[:, g * FSH : (g + 1) * FSH],
                        in1=zrep[:],
                        scale=1.0,
                        bias=0.0,
                    )
                # drain PE-down psum on scalar engine
                for dt in range(DP // 512):
                    sl = slice(dt * 512, (dt + 1) * 512)
                    nc.scalar.copy(osbp[0:1, sl], pdp[0:1, sl])

            nc.sync.dma_start(out=outp_d.ap(), in_=osbp[:])
            nc.sync.dma_start(out=outd_d.ap(), in_=outd_sb[:])

    nc.compile()
    return nc


def _get_nc():
    if "nc" not in _CACHE:
        _CACHE["nc"] = _build_nc()
    return _CACHE["nc"]


def make_in_maps(x, Wup, Wgatet, Wdownt, threshold):
    """Shard full inputs into the 8 per-core input maps (bf16 weights)."""
    x_flat = np.asarray(x, dtype=np.float32).reshape(D)
    xcol = np.ascontiguousarray(x_flat.reshape(NCD, 128).T).astype(BF)
    xf = x_flat.astype(BF)
    thr = np.asarray(threshold, dtype=np.float32).reshape(1)
    Wup = np.asarray(Wup, dtype=np.float32)
    Wgatet = np.asarray(Wgatet, dtype=np.float32)
    Wdownt = np.asarray(Wdownt, dtype=np.float32)
    in_maps = []
    for i in range(NCORES):
        sl = slice(i * FSH, (i + 1) * FSH)
        wg_slice = Wgatet[:, sl]                  # [D, FSH] d-major
        wu_slice = Wup[sl, :]                     # [FSH, D] f-major
        wd_slice = Wdownt[sl, :]                  # [FSH, D] f-major

        wgT = np.ascontiguousarray(wg_slice.T)    # [FSH, D] f-major
        wgd = (
            wgT[:FD].reshape(FDC, 128, D).transpose(1, 0, 2).reshape(128, FDC * D)
        )
        wud = (
            wu_slice[:FD]
            .reshape(FDC, 128, D)
            .transpose(1, 0, 2)
            .reshape(128, FDC * D)
        )
        wgp = (
            wg_slice[:, FD:]
            .reshape(NCD, 128, FP)
            .transpose(1, 0, 2)
            .reshape(128, NCD * FP)
        )
        wuT = np.ascontiguousarray(wu_slice.T)    # [D, FSH] d-major
        wup = (
            wuT[:, FD:]
            .reshape(NCD, 128, FP)
            .transpose(1, 0, 2)
            .reshape(128, NCD * FP)
        )
        wd_pad = np.zeros((NCF * 128, DP), dtype=np.float32)
        wd_pad[:FSH] = wd_slice[:, :DP]
        wdp = (
            wd_pad.reshape(NCF, 128, DP).transpose(1, 0, 2).reshape(128, NCF * DP)
        )
        wdT = np.ascontiguousarray(wd_slice.T)    # [D, FSH] d-major
        wdd = (
            wdT[DP:]
            .reshape(NDVG, 128, FSH)
            .transpose(1, 0, 2)
            .reshape(128, NDVG * FSH)
        )
        in_maps.append(
            {
                "xc": xcol,
                "xf": xf,
                "wgd": np.ascontiguousarray(wgd).astype(BF),
                "wud": np.ascontiguousarray(wud).astype(BF),
                "wgp": np.ascontiguousarray(wgp).astype(BF),
                "wup": np.ascontiguousarray(wup).astype(BF),
                "wdp": np.ascontiguousarray(wdp).astype(BF),
                "wdd": np.ascontiguousarray(wdd).astype(BF),
                "thr": thr,
            }
        )
    return in_maps


def run_sharded(x, Wup, Wgatet, Wdownt, threshold, trace=False, tmpdir=None):
    """Run on the 8 NeuronCores; returns (full_output, BassKernelResults)."""
    nc = _get_nc()
    in_maps = make_in_maps(x, Wup, Wgatet, Wdownt, threshold)
    res = run_bass_kernel_spmd(
        nc, in_maps, list(range(NCORES)), trace=trace, tmpdir=tmpdir
    )
    acc = np.zeros(D, dtype=np.float64)
    for r in res.results:
        acc[:DP] += r["outp"].reshape(DP).astype(np.float64)
        acc[DP:] += r["outd"].T.reshape(D - DP).astype(np.float64)
    out = acc.astype(np.float32).reshape(1, 1, D)
    return out, res


def kernel(x, Wup, Wgatet, Wdownt, threshold):
    out, _ = run_sharded(x, Wup, Wgatet, Wdownt, threshold)
    return out


# revision 15
# speedup vs baseline: 1.6646x; 1.0302x over previous
# CATS-SwiGLU decode kernel for TRN2 (8 NeuronCores, SPMD tensor-parallel).
# v4: bf16 weights streamed to BOTH the TensorEngine and the Vector engine
# concurrently (each sustains ~120-130 Gelem/s; combined they outrun the
# ~430 GB/s per-core DMA fabric).  Three HWDGE queues: sync + gpsimd carry
# the gate/up stream (alternating pieces, deep prefetch pools so the DMA
# runs ahead of compute), and the scalar queue carries the down-projection
# stream, gated behind the gate matrix by a 2-byte SBUF->SBUF dummy DMA
# reading the gate's last DVE accumulator (the Tile list scheduler cannot
# hoist it, and the ACT engine stalling on the gate is harmless).
#
#   x1    = silu(x @ Wgatet); flags = |x1| > thr
#   z     = where(flags, (x @ Wup.T) * x1, 0);  out = z @ Wdownt
#
# f-split for gate/up: f-rows [0:FD) via DVE affine_mul_reduce over f-major
# tiles [128f, 4096d] (accumulator lands f-on-partitions, already
# transposed for the down GEMV); f-rows [FD:1376) via PE accumulating GEMV
# matmuls (stationary x column, weights moving).  d-split for down:
# d-cols [0:DP) on PE (f-major weights, z-column stationary), d-cols
# [DP:4096) on DVE (d-major weights times a ones-broadcast z).
import sys

for _p in ("/opt/trn_rl_repo",):
    if _p not in sys.path:
        sys.path.insert(0, _p)

import numpy as np
import ml_dtypes

import concourse.bass as bass
import concourse.tile as tile
from concourse import bacc, mybir
from concourse.bass_utils import run_bass_kernel_spmd
from concourse.masks import make_identity

D = 4096
FF = 11008
NCORES = 8
FSH = FF // NCORES            # 1376 rows of d_ff per core
NCD = D // 128                # 32 contraction chunks for gate/up PE part
NCF = (FSH + 127) // 128      # 11 f chunks
LASTF = FSH - 128 * (NCF - 1)  # 96 rows in the last f chunk

FDC = 6                       # f chunks on DVE for gate/up
FD = FDC * 128                # 768
FP = FSH - FD                 # 608 PE-part f width
FTP = ((0, 512), (512, FP - 512))   # PE-part psum f-tiles
NPP = 4                       # PE-part stream pieces per matrix (8 chunks)
CPP = NCD // NPP              # 8 chunks per PE piece
PPW = CPP * FP                # PE piece width (elems/partition)
PBUFS = 3                     # PE-piece prefetch depth
TBUFS = 5                     # DVE-tile prefetch depth

DPC = 16                      # d-cols on PE for down, in units of 128
DP = DPC * 128                # 2048
NDVG = (D - DP) // 128        # 16 DVE down groups
WDPB = (3, 3, 3, 2)           # wdp stream pieces, in f-chunks
WDDB = (4, 4, 4, 4)           # wdd stream pieces, in d-groups

F32 = mybir.dt.float32
BF16 = mybir.dt.bfloat16
BF = ml_dtypes.bfloat16

_CACHE = {}


def _bcast(ap, parts):
    return bass.AP(tensor=ap.tensor, offset=ap.offset, ap=[[0, parts]] + list(ap.ap))


def _build_nc():
    nc = bacc.Bacc("TRN2", target_bir_lowering=False, debug=False)

    xc_d = nc.dram_tensor("xc", [128, NCD], BF16, kind="ExternalInput")
    xf_d = nc.dram_tensor("xf", [D], BF16, kind="ExternalInput")
    wgd_d = nc.dram_tensor("wgd", [128, FDC * D], BF16, kind="ExternalInput")
    wud_d = nc.dram_tensor("wud", [128, FDC * D], BF16, kind="ExternalInput")
    wgp_d = nc.dram_tensor("wgp", [128, NCD * FP], BF16, kind="ExternalInput")
    wup_d = nc.dram_tensor("wup", [128, NCD * FP], BF16, kind="ExternalInput")
    wdp_d = nc.dram_tensor("wdp", [128, NCF * DP], BF16, kind="ExternalInput")
    wdd_d = nc.dram_tensor("wdd", [128, NDVG * FSH], BF16, kind="ExternalInput")
    thr_d = nc.dram_tensor("thr", [1], F32, kind="ExternalInput")
    outp_d = nc.dram_tensor("outp", [DP], F32, kind="ExternalOutput")
    outd_d = nc.dram_tensor("outd", [128, NDVG], F32, kind="ExternalOutput")

    with tile.TileContext(nc) as tc:
        with (
            tc.tile_pool(name="const", bufs=1) as cp,
            tc.tile_pool(name="ppool", bufs=PBUFS) as ppool,
            tc.tile_pool(name="tpool", bufs=TBUFS) as tpool,
            tc.tile_pool(name="acts", bufs=1) as acts,
        ):
            # constants ride the gpsimd ring ahead of its weight pieces
            xcol = cp.tile([128, NCD], BF16)
            nc.gpsimd.dma_start(out=xcol[:], in_=xc_d.ap())
            xrep = cp.tile([128, D], BF16)
            nc.gpsimd.dma_start(out=xrep[:], in_=_bcast(xf_d.ap(), 128))
            thr_sb = cp.tile([128, 1], F32)
            nc.gpsimd.dma_start(out=thr_sb[:], in_=_bcast(thr_d.ap(), 128))
            ones_col = cp.tile([1, 128], BF16)
            nc.vector.memset(ones_col[:], 1.0)
            ident = cp.tile([128, 128], BF16)
            make_identity(nc, ident[:])

            # activation scratch
            accg = acts.tile([128, FDC], F32)   # DVE-part gate accum
            accu = acts.tile([128, FDC], F32)   # DVE-part up accum
            sgd = acts.tile([128, FDC], F32)
            x1d = acts.tile([128, FDC], F32)
            abd = acts.tile([128, FDC], F32)
            mkd = acts.tile([128, FDC], F32)
            xmd = acts.tile([128, FDC], F32)
            sgp = acts.tile([1, FP], F32)
            x1p = acts.tile([1, FP], F32)
            abp = acts.tile([1, FP], F32)
            mkp = acts.tile([1, FP], F32)
            xmp = acts.tile([1, FP], F32)
            z_row = acts.tile([1, NCF * 128], BF16)
            nc.vector.memset(z_row[:], 0.0)
            z_bf = acts.tile([128, NCF], BF16)
            nc.vector.memset(z_bf[:], 0.0)
            zrep = acts.tile([128, FSH], BF16)
            osbp = acts.tile([1, DP], F32)
            outd_sb = acts.tile([128, NDVG], F32)
            warm = acts.tile([128, 1], F32)
            nc.scalar.activation(
                warm[:], thr_sb[:], mybir.ActivationFunctionType.Sigmoid
            )
            nc.scalar.activation(
                warm[:], thr_sb[:], mybir.ActivationFunctionType.Abs
            )
            nc.scalar.copy(warm[:], thr_sb[:])

            # resident down-weight tiles; their DMAs ride the scalar queue,
            # pinned behind the gate matrix by strided dummy DMAs
            wdp_sb = acts.tile([128, NCF * DP], BF16)
            wdd_sb = acts.tile([128, NDVG * FSH], BF16)

            qs = (nc.sync, nc.gpsimd)
            nq = {"i": 0}

            def next_q():
                q = qs[nq["i"] % 2]
                nq["i"] += 1
                return q

            # ---- gate/up weight stream ----
            ORDER = ("P", "T", "T", "P", "T", "T", "P", "T", "T", "P")
            ptiles = {}
            dtiles = {}

            def emit_piece(mi, kind, idx):
                if kind == "P":
                    dram = wgp_d if mi == 0 else wup_d
                    t = ppool.tile([128, PPW], BF16, tag="pw", name="pw")
                    next_q().dma_start(
                        out=t[:], in_=dram.ap()[:, idx * PPW : (idx + 1) * PPW]
                    )
                    ptiles[(mi, idx)] = t
                else:
                    dram = wgd_d if mi == 0 else wud_d
                    t = tpool.tile([128, D], BF16, tag="tw", name="tw")
                    next_q().dma_start(
                        out=t[:], in_=dram.ap()[:, idx * D : (idx + 1) * D]
                    )
                    dtiles[(mi, idx)] = t

            stream = []
            for mi in range(2):
                cnt = {"P": 0, "T": 0}
                for kind in ORDER:
                    stream.append((mi, kind, cnt[kind]))
                    cnt[kind] += 1

            with tc.tile_pool(name="ps1", bufs=1, space="PSUM") as ps1:
                pgp = ps1.tile([1, 1024], F32)
                pup = ps1.tile([1, 1024], F32)
                pzrow = ps1.tile([1, FDC * 128], F32)
                pzcol = ps1.tile([128, NCF - FDC], F32)

                # greedy prefetch bounded by per-kind pool depth
                emitted = 0
                inflight = {"P": 0, "T": 0}
                bufs = {"P": PBUFS, "T": TBUFS}

                def pump():
                    nonlocal emitted
                    while emitted < len(stream):
                        mi, kind, idx = stream[emitted]
                        if inflight[kind] >= bufs[kind]:
                            return
                        emit_piece(mi, kind, idx)
                        inflight[kind] += 1
                        emitted += 1

                def consume(mi, kind, idx):
                    if kind == "P":
                        accp = pgp if mi == 0 else pup
                        t = ptiles[(mi, idx)]
                        for cc in range(CPP):
                            c = idx * CPP + cc
                            for toff, tlen in FTP:
                                nc.tensor.matmul(
                                    out=accp[0:1, toff : toff + tlen],
                                    lhsT=xcol[:, c : c + 1],
                                    rhs=t[:, cc * FP + toff : cc * FP + toff + tlen],
                                    start=(c == 0),
                                    stop=(c == NCD - 1),
                                )
                    else:
                        acct = accg if mi == 0 else accu
                        t = dtiles[(mi, idx)]
                        nc.vector.affine_mul_reduce(
                            out=t[:],
                            accum_out=acct[:, idx : idx + 1],
                            in0=t[:],
                            in1=xrep[:],
                            scale=1.0,
                            bias=0.0,
                        )

                def emit_wd_dmas():
                    # dummy DMAs touching the first element of each wd
                    # stream piece, reading the gate's last DVE tile: the
                    # WAW dependency pins every wd piece DMA behind the
                    # gate matrix, so the list scheduler cannot hoist them.
                    trig = dtiles[(0, FDC - 1)]
                    nc.scalar.dma_start(
                        out=wdp_sb[0:1, 0 : 9 * DP + 1 : 3 * DP],
                        in_=trig[0:1, 0:4],
                    )
                    nc.scalar.dma_start(
                        out=wdd_sb[0:1, 0 : 12 * FSH + 1 : 4 * FSH],
                        in_=trig[0:1, 4:8],
                    )
                    po = do = 0
                    for k in range(4):
                        pw = WDPB[k] * DP
                        nc.scalar.dma_start(
                            out=wdp_sb[:, po * DP : po * DP + pw],
                            in_=wdp_d.ap()[:, po * DP : po * DP + pw],
                        )
                        po += WDPB[k]
                        dw = WDDB[k] * FSH
                        nc.scalar.dma_start(
                            out=wdd_sb[:, do * FSH : do * FSH + dw],
                            in_=wdd_d.ap()[:, do * FSH : do * FSH + dw],
                        )
                        do += WDDB[k]

                pump()
                for k, item in enumerate(stream):
                    consume(*item)
                    inflight[item[1]] -= 1
                    pump()
                    if k == len(stream) - 1:
                        emit_wd_dmas()

                # ---- gate elementwise ----
                nc.scalar.activation(
                    sgd[:], accg[:], mybir.ActivationFunctionType.Sigmoid
                )
                nc.vector.tensor_mul(x1d[:], accg[:], sgd[:])
                nc.scalar.activation(
                    abd[:], x1d[:], mybir.ActivationFunctionType.Abs
                )
                nc.vector.tensor_scalar(
                    out=mkd[:], in0=abd[:], scalar1=thr_sb[:], scalar2=None,
                    op0=mybir.AluOpType.is_gt,
                )
                nc.vector.tensor_mul(xmd[:], x1d[:], mkd[:])
                nc.scalar.activation(
                    sgp[:], pgp[0:1, 0:FP], mybir.ActivationFunctionType.Sigmoid
                )
                nc.vector.tensor_mul(x1p[:], pgp[0:1, 0:FP], sgp[:])
                nc.scalar.activation(
                    abp[:], x1p[:], mybir.ActivationFunctionType.Abs
                )
                nc.vector.tensor_scalar(
                    out=mkp[:], in0=abp[:], scalar1=thr_sb[0:1, :], scalar2=None,
                    op0=mybir.AluOpType.is_gt,
                )
                nc.vector.tensor_mul(xmp[:], x1p[:], mkp[:])

                # ---- z ----
                nc.vector.tensor_mul(z_bf[:, 0:FDC], accu[:], xmd[:])
                nc.vector.tensor_mul(
                    z_row[0:1, FD:FSH], pup[0:1, 0:FP], xmp[:]
                )
                # DVE-part z to row form (for the z broadcast)
                for c in range(FDC):
                    nc.tensor.matmul(
                        out=pzrow[0:1, c * 128 : (c + 1) * 128],
                        lhsT=z_bf[:, c : c + 1],
                        rhs=ident[:],
                        start=True,
                        stop=True,
                    )
                nc.scalar.copy(z_row[0:1, 0:FD], pzrow[0:1, 0:FD])
                # PE-part z to column form
                for c in range(FDC, NCF):
                    pc = 128 if c < NCF - 1 else LASTF
                    nc.tensor.matmul(
                        out=pzcol[0:pc, c - FDC : c - FDC + 1],
                        lhsT=z_row[0:1, c * 128 : c * 128 + pc],
                        rhs=ones_col[0:1, 0:1],
                        start=True,
                        stop=True,
                    )
                nc.scalar.copy(z_bf[:, FDC:NCF], pzcol[:, 0 : NCF - FDC])

            with tc.tile_pool(name="ps2", bufs=1, space="PSUM") as ps2:
                pdp = ps2.tile([1, DP], F32)
                przep = ps2.tile([128, 1536], F32)
                for toff, tlen in ((0, 512), (512, 512), (1024, FSH - 1024)):
                    nc.tensor.matmul(
                        out=przep[:, toff : toff + tlen],
                        lhsT=ones_col[:],
                        rhs=z_row[0:1, toff : toff + tlen],
                        start=True,
                        stop=True,
                    )
                nc.scalar.copy(zrep[:], przep[:, 0:FSH])

                # PE and DVE down parts
                for c in range(NCF):
                    pc = 128 if c < NCF - 1 else LASTF
                    for dt in range(DP // 512):
                        nc.tensor.matmul(
                            out=pdp[0:1, dt * 512 : (dt + 1) * 512],
                            lhsT=z_bf[0:pc, c : c + 1],
                            rhs=wdp_sb[0:pc, c * DP + dt * 512 : c * DP + (dt + 1) * 512],
                            start=(c == 0),
                            stop=(c == NCF - 1),
                        )
                for g in range(NDVG):
                    nc.vector.affine_mul_reduce(
                        out=wdd_sb[:, g * FSH : (g + 1) * FSH],
                        accum_out=outd_sb[:, g : g + 1],
                        in0=wdd_sb[:, g * FSH : (g + 1) * FSH],
                        in1=zrep[:],
                        scale=1.0,
                        bias=0.0,
                    )
                for dt in range(DP // 512):
                    sl = slice(dt * 512, (dt + 1) * 512)
                    nc.scalar.copy(osbp[0:1, sl], pdp[0:1, sl])

            nc.sync.dma_start(out=outp_d.ap(), in_=osbp[:])
            nc.sync.dma_start(out=outd_d.ap(), in_=outd_sb[:])

    nc.compile()
    return nc


def _get_nc():
    if "nc" not in _CACHE:
        _CACHE["nc"] = _build_nc()
    return _CACHE["nc"]


def make_in_maps(x, Wup, Wgatet, Wdownt, threshold):
    """Shard full inputs into the 8 per-core input maps (bf16 weights)."""
    x_flat = np.asarray(x, dtype=np.float32).reshape(D)
    xcol = np.ascontiguousarray(x_flat.reshape(NCD, 128).T).astype(BF)
    xf = x_flat.astype(BF)
    thr = np.asarray(threshold, dtype=np.float32).reshape(1)
    Wup = np.asarray(Wup, dtype=np.float32)
    Wgatet = np.asarray(Wgatet, dtype=np.float32)
    Wdownt = np.asarray(Wdownt, dtype=np.float32)
    in_maps = []
    for i in range(NCORES):
        sl = slice(i * FSH, (i + 1) * FSH)
        wg_slice = Wgatet[:, sl]                  # [D, FSH] d-major
        wu_slice = Wup[sl, :]                     # [FSH, D] f-major
        wd_slice = Wdownt[sl, :]                  # [FSH, D] f-major

        wgT = np.ascontiguousarray(wg_slice.T)    # [FSH, D] f-major
        wgd = (
            wgT[:FD].reshape(FDC, 128, D).transpose(1, 0, 2).reshape(128, FDC * D)
        )
        wud = (
            wu_slice[:FD]
            .reshape(FDC, 128, D)
            .transpose(1, 0, 2)
            .reshape(128, FDC * D)
        )
        wgp = (
            wg_slice[:, FD:]
            .reshape(NCD, 128, FP)
            .transpose(1, 0, 2)
            .reshape(128, NCD * FP)
        )
        wuT = np.ascontiguousarray(wu_slice.T)    # [D, FSH] d-major
        wup = (
            wuT[:, FD:]
            .reshape(NCD, 128, FP)
            .transpose(1, 0, 2)
            .reshape(128, NCD * FP)
        )
        wd_pad = np.zeros((NCF * 128, DP), dtype=np.float32)
        wd_pad[:FSH] = wd_slice[:, :DP]
        wdp = (
            wd_pad.reshape(NCF, 128, DP).transpose(1, 0, 2).reshape(128, NCF * DP)
        )
        wdT = np.ascontiguousarray(wd_slice.T)    # [D, FSH] d-major
        wdd = (
            wdT[DP:]
            .reshape(NDVG, 128, FSH)
            .transpose(1, 0, 2)
            .reshape(128, NDVG * FSH)
        )
        in_maps.append(
            {
                "xc": xcol,
                "xf": xf,
                "wgd": np.ascontiguousarray(wgd).astype(BF),
                "wud": np.ascontiguousarray(wud).astype(BF),
                "wgp": np.ascontiguousarray(wgp).astype(BF),
                "wup": np.ascontiguousarray(wup).astype(BF),
                "wdp": np.ascontiguousarray(wdp).astype(BF),
                "wdd": np.ascontiguousarray(wdd).astype(BF),
                "thr": thr,
            }
        )
    return in_maps


def run_sharded(x, Wup, Wgatet, Wdownt, threshold, trace=False, tmpdir=None):
    """Run on the 8 NeuronCores; returns (full_output, BassKernelResults)."""
    nc = _get_nc()
    in_maps = make_in_maps(x, Wup, Wgatet, Wdownt, threshold)
    res = run_bass_kernel_spmd(
        nc, in_maps, list(range(NCORES)), trace=trace, tmpdir=tmpdir
    )
    acc = np.zeros(D, dtype=np.float64)
    for r in res.results:
        acc[:DP] += r["outp"].reshape(DP).astype(np.float64)
        acc[DP:] += r["outd"].T.reshape(D - DP).astype(np.float64)
    out = acc.astype(np.float32).reshape(1, 1, D)
    return out, res


def kernel(x, Wup, Wgatet, Wdownt, threshold):
    out, _ = run_sharded(x, Wup, Wgatet, Wdownt, threshold)
    return out


# revision 20
# speedup vs baseline: 1.9464x; 1.1693x over previous
# CATS-SwiGLU decode kernel for TRN2 (8 NeuronCores, SPMD tensor-parallel).
# v4: bf16 weights streamed to BOTH the TensorEngine and the Vector engine
# concurrently (each sustains ~120-130 Gelem/s; combined they outrun the
# ~430 GB/s per-core DMA fabric).  Three HWDGE queues: sync + gpsimd carry
# the gate/up stream (alternating pieces, deep prefetch pools so the DMA
# runs ahead of compute), and the scalar queue carries the down-projection
# stream, gated behind the gate matrix by a 2-byte SBUF->SBUF dummy DMA
# reading the gate's last DVE accumulator (the Tile list scheduler cannot
# hoist it, and the ACT engine stalling on the gate is harmless).
#
#   x1    = silu(x @ Wgatet); flags = |x1| > thr
#   z     = where(flags, (x @ Wup.T) * x1, 0);  out = z @ Wdownt
#
# f-split for gate/up: f-rows [0:FD) via DVE affine_mul_reduce over f-major
# tiles [128f, 4096d] (accumulator lands f-on-partitions, already
# transposed for the down GEMV); f-rows [FD:1376) via PE accumulating GEMV
# matmuls (stationary x column, weights moving).  d-split for down:
# d-cols [0:DP) on PE (f-major weights, z-column stationary), d-cols
# [DP:4096) on DVE (d-major weights times a ones-broadcast z).
import sys

for _p in ("/opt/trn_rl_repo",):
    if _p not in sys.path:
        sys.path.insert(0, _p)

import numpy as np
import ml_dtypes

import concourse.bass as bass
import concourse.tile as tile
from concourse import bacc, mybir
from concourse.bass_utils import run_bass_kernel_spmd
from concourse.masks import make_identity

D = 4096
FF = 11008
NCORES = 8
FSH = FF // NCORES            # 1376 rows of d_ff per core
NCD = D // 128                # 32 contraction chunks for gate/up PE part
NCF = (FSH + 127) // 128      # 11 f chunks
LASTF = FSH - 128 * (NCF - 1)  # 96 rows in the last f chunk

FDC = 6                       # f chunks on DVE for gate/up
FD = FDC * 128                # 768
FP = FSH - FD                 # 608 PE-part f width
FTP = ((0, 512), (512, FP - 512))   # PE-part psum f-tiles
NPP = 2                       # PE-part stream pieces per matrix (16 chunks)
CPP = NCD // NPP              # 16 chunks per PE piece
PPW = CPP * FP                # PE piece width (elems/partition)
CPT = 2                       # DVE chunks per stream tile (16KB rows)
NPT = FDC // CPT              # 3 DVE tiles per matrix
PBUFS = 2                     # PE-piece prefetch depth
TBUFS = 2                     # DVE-tile prefetch depth

DPC = 16                      # d-cols on PE for down, in units of 128
DP = DPC * 128                # 2048
NDVG = (D - DP) // 128        # 16 DVE down groups
WDPB = (3, 3, 3, 2)           # wdp stream pieces, in f-chunks
WDDB = (4, 4, 4, 4)           # wdd stream pieces, in d-groups

F32 = mybir.dt.float32
BF16 = mybir.dt.bfloat16
BF = ml_dtypes.bfloat16

_CACHE = {}


def _bcast(ap, parts):
    return bass.AP(tensor=ap.tensor, offset=ap.offset, ap=[[0, parts]] + list(ap.ap))


def _build_nc():
    nc = bacc.Bacc("TRN2", target_bir_lowering=False, debug=False)

    xc_d = nc.dram_tensor("xc", [128, NCD], BF16, kind="ExternalInput")
    xf_d = nc.dram_tensor("xf", [D], BF16, kind="ExternalInput")
    wgd_d = nc.dram_tensor("wgd", [128, FDC * D], BF16, kind="ExternalInput")
    wud_d = nc.dram_tensor("wud", [128, FDC * D], BF16, kind="ExternalInput")
    wgp_d = nc.dram_tensor("wgp", [128, NCD * FP], BF16, kind="ExternalInput")
    wup_d = nc.dram_tensor("wup", [128, NCD * FP], BF16, kind="ExternalInput")
    wdp_d = nc.dram_tensor("wdp", [128, NCF * DP], BF16, kind="ExternalInput")
    wdd_d = nc.dram_tensor("wdd", [128, NDVG * FSH], BF16, kind="ExternalInput")
    thr_d = nc.dram_tensor("thr", [1], F32, kind="ExternalInput")
    outp_d = nc.dram_tensor("outp", [DP], F32, kind="ExternalOutput")
    outd_d = nc.dram_tensor("outd", [128, NDVG], F32, kind="ExternalOutput")

    with tile.TileContext(nc) as tc:
        with (
            tc.tile_pool(name="const", bufs=1) as cp,
            tc.tile_pool(name="ppool", bufs=PBUFS) as ppool,
            tc.tile_pool(name="tpool", bufs=TBUFS) as tpool,
            tc.tile_pool(name="acts", bufs=1) as acts,
        ):
            # constants ride the gpsimd ring ahead of its weight pieces
            xcol = cp.tile([128, NCD], BF16)
            nc.gpsimd.dma_start(out=xcol[:], in_=xc_d.ap())
            xrep = cp.tile([128, D], BF16)
            nc.gpsimd.dma_start(out=xrep[:], in_=_bcast(xf_d.ap(), 128))
            thr_sb = cp.tile([128, 1], F32)
            nc.gpsimd.dma_start(out=thr_sb[:], in_=_bcast(thr_d.ap(), 128))
            ones_col = cp.tile([1, 128], BF16)
            nc.vector.memset(ones_col[:], 1.0)
            ident = cp.tile([128, 128], BF16)
            make_identity(nc, ident[:])

            # activation scratch
            accg = acts.tile([128, FDC], F32)   # DVE-part gate accum
            accu = acts.tile([128, FDC], F32)   # DVE-part up accum
            sgd = acts.tile([128, FDC], F32)
            x1d = acts.tile([128, FDC], F32)
            abd = acts.tile([128, FDC], F32)
            mkd = acts.tile([128, FDC], F32)
            xmd = acts.tile([128, FDC], F32)
            sgp = acts.tile([1, FP], F32)
            x1p = acts.tile([1, FP], F32)
            abp = acts.tile([1, FP], F32)
            mkp = acts.tile([1, FP], F32)
            xmp = acts.tile([1, FP], F32)
            z_row = acts.tile([1, NCF * 128], BF16)
            nc.vector.memset(z_row[:], 0.0)
            z_bf = acts.tile([128, NCF], BF16)
            nc.vector.memset(z_bf[:], 0.0)
            zrep = acts.tile([128, FSH], BF16)
            osbp = acts.tile([1, DP], F32)
            outd_sb = acts.tile([128, NDVG], F32)
            warm = acts.tile([128, 1], F32)
            nc.scalar.activation(
                warm[:], thr_sb[:], mybir.ActivationFunctionType.Sigmoid
            )
            nc.scalar.activation(
                warm[:], thr_sb[:], mybir.ActivationFunctionType.Abs
            )
            nc.scalar.copy(warm[:], thr_sb[:])

            # resident down-weight tiles; their DMAs ride the scalar queue,
            # pinned behind the gate matrix by strided dummy DMAs
            wdp_sb = acts.tile([128, NCF * DP], BF16)
            wdd_sb = acts.tile([128, NDVG * FSH], BF16)

            # ---- gate/up weight stream ----
            # per-matrix piece order and queue map, byte-balanced so the
            # gate matrix fully lands before the up matrix needs bandwidth
            ORDER = (("P", 0), ("T", 0), ("T", 1), ("P", 1), ("T", 2))
            QMAP = {0: (0, 1, 1, 0, 1), 1: (1, 0, 0, 1, 0)}
            qs = (nc.sync, nc.gpsimd)
            ptiles = {}
            dtiles = {}

            def emit_piece(mi, kind, idx, oi):
                q = qs[QMAP[mi][oi]]
                if kind == "P":
                    dram = wgp_d if mi == 0 else wup_d
                    t = ppool.tile([128, PPW], BF16, tag="pw", name="pw")
                    q.dma_start(
                        out=t[:], in_=dram.ap()[:, idx * PPW : (idx + 1) * PPW]
                    )
                    ptiles[(mi, idx)] = t
                else:
                    dram = wgd_d if mi == 0 else wud_d
                    t = tpool.tile([128, CPT * D], BF16, tag="tw", name="tw")
                    q.dma_start(
                        out=t[:],
                        in_=dram.ap()[:, idx * CPT * D : (idx + 1) * CPT * D],
                    )
                    dtiles[(mi, idx)] = t

            stream = []
            for mi in range(2):
                for oi, (kind, idx) in enumerate(ORDER):
                    stream.append((mi, kind, idx, oi))

            with tc.tile_pool(name="ps1", bufs=1, space="PSUM") as ps1:
                pgp = ps1.tile([1, 1024], F32)
                pup = ps1.tile([1, 1024], F32)
                pzrow = ps1.tile([1, FDC * 128], F32)
                pzcol = ps1.tile([128, NCF - FDC], F32)

                # greedy prefetch bounded by per-kind pool depth
                emitted = 0
                inflight = {"P": 0, "T": 0}
                bufs = {"P": PBUFS, "T": TBUFS}

                def pump():
                    nonlocal emitted
                    while emitted < len(stream):
                        mi, kind, idx, oi = stream[emitted]
                        if inflight[kind] >= bufs[kind]:
                            return
                        emit_piece(mi, kind, idx, oi)
                        inflight[kind] += 1
                        emitted += 1

                def consume(mi, kind, idx, oi):
                    if kind == "P":
                        accp = pgp if mi == 0 else pup
                        t = ptiles[(mi, idx)]
                        for cc in range(CPP):
                            c = idx * CPP + cc
                            for toff, tlen in FTP:
                                nc.tensor.matmul(
                                    out=accp[0:1, toff : toff + tlen],
                                    lhsT=xcol[:, c : c + 1],
                                    rhs=t[:, cc * FP + toff : cc * FP + toff + tlen],
                                    start=(c == 0),
                                    stop=(c == NCD - 1),
                                )
                    else:
                        acct = accg if mi == 0 else accu
                        t = dtiles[(mi, idx)]
                        for j in range(CPT):
                            c = idx * CPT + j
                            nc.vector.affine_mul_reduce(
                                out=t[:, j * D : (j + 1) * D],
                                accum_out=acct[:, c : c + 1],
                                in0=t[:, j * D : (j + 1) * D],
                                in1=xrep[:],
                                scale=1.0,
                                bias=0.0,
                            )

                def emit_wd_dmas():
                    # dummy DMAs touching the first element of each wd
                    # stream piece, reading the up matrix's second-to-last
                    # DVE tile: the WAW dependency pins every wd piece DMA
                    # behind the bulk of the gate/up stream, so the list
                    # scheduler cannot hoist them into it.
                    trig = dtiles[(1, NPT - 2)]
                    nc.scalar.dma_start(
                        out=wdp_sb[0:1, 0 : 9 * DP + 1 : 3 * DP],
                        in_=trig[0:1, 0:4],
                    )
                    nc.scalar.dma_start(
                        out=wdd_sb[0:1, 0 : 12 * FSH + 1 : 4 * FSH],
                        in_=trig[0:1, 4:8],
                    )
                    po = do = 0
                    for k in range(4):
                        pw = WDPB[k] * DP
                        nc.scalar.dma_start(
                            out=wdp_sb[:, po * DP : po * DP + pw],
                            in_=wdp_d.ap()[:, po * DP : po * DP + pw],
                        )
                        po += WDPB[k]
                        dw = WDDB[k] * FSH
                        nc.scalar.dma_start(
                            out=wdd_sb[:, do * FSH : do * FSH + dw],
                            in_=wdd_d.ap()[:, do * FSH : do * FSH + dw],
                        )
                        do += WDDB[k]

                pump()
                for k, item in enumerate(stream):
                    consume(*item)
                    inflight[item[1]] -= 1
                    pump()
                    if k == len(stream) - 1:
                        emit_wd_dmas()

                # ---- gate elementwise ----
                nc.scalar.activation(
                    sgd[:], accg[:], mybir.ActivationFunctionType.Sigmoid
                )
                nc.vector.tensor_mul(x1d[:], accg[:], sgd[:])
                nc.scalar.activation(
                    abd[:], x1d[:], mybir.ActivationFunctionType.Abs
                )
                nc.vector.tensor_scalar(
                    out=mkd[:], in0=abd[:], scalar1=thr_sb[:], scalar2=None,
                    op0=mybir.AluOpType.is_gt,
                )
                nc.vector.tensor_mul(xmd[:], x1d[:], mkd[:])
                nc.scalar.activation(
                    sgp[:], pgp[0:1, 0:FP], mybir.ActivationFunctionType.Sigmoid
                )
                nc.vector.tensor_mul(x1p[:], pgp[0:1, 0:FP], sgp[:])
                nc.scalar.activation(
                    abp[:], x1p[:], mybir.ActivationFunctionType.Abs
                )
                nc.vector.tensor_scalar(
                    out=mkp[:], in0=abp[:], scalar1=thr_sb[0:1, :], scalar2=None,
                    op0=mybir.AluOpType.is_gt,
                )
                nc.vector.tensor_mul(xmp[:], x1p[:], mkp[:])

                # ---- z ----
                nc.vector.tensor_mul(z_bf[:, 0:FDC], accu[:], xmd[:])
                nc.vector.tensor_mul(
                    z_row[0:1, FD:FSH], pup[0:1, 0:FP], xmp[:]
                )
                # DVE-part z to row form (for the z broadcast)
                for c in range(FDC):
                    nc.tensor.matmul(
                        out=pzrow[0:1, c * 128 : (c + 1) * 128],
                        lhsT=z_bf[:, c : c + 1],
                        rhs=ident[:],
                        start=True,
                        stop=True,
                    )
                nc.scalar.copy(z_row[0:1, 0:FD], pzrow[0:1, 0:FD])
                # PE-part z to column form
                for c in range(FDC, NCF):
                    pc = 128 if c < NCF - 1 else LASTF
                    nc.tensor.matmul(
                        out=pzcol[0:pc, c - FDC : c - FDC + 1],
                        lhsT=z_row[0:1, c * 128 : c * 128 + pc],
                        rhs=ones_col[0:1, 0:1],
                        start=True,
                        stop=True,
                    )
                nc.scalar.copy(z_bf[:, FDC:NCF], pzcol[:, 0 : NCF - FDC])

            with tc.tile_pool(name="ps2", bufs=1, space="PSUM") as ps2:
                pdp = ps2.tile([1, DP], F32)
                przep = ps2.tile([128, 1536], F32)
                for toff, tlen in ((0, 512), (512, 512), (1024, FSH - 1024)):
                    nc.tensor.matmul(
                        out=przep[:, toff : toff + tlen],
                        lhsT=ones_col[:],
                        rhs=z_row[0:1, toff : toff + tlen],
                        start=True,
                        stop=True,
                    )
                nc.scalar.copy(zrep[:], przep[:, 0:FSH])

                # PE and DVE down parts
                for c in range(NCF):
                    pc = 128 if c < NCF - 1 else LASTF
                    for dt in range(DP // 512):
                        nc.tensor.matmul(
                            out=pdp[0:1, dt * 512 : (dt + 1) * 512],
                            lhsT=z_bf[0:pc, c : c + 1],
                            rhs=wdp_sb[0:pc, c * DP + dt * 512 : c * DP + (dt + 1) * 512],
                            start=(c == 0),
                            stop=(c == NCF - 1),
                        )
                for g in range(NDVG):
                    nc.vector.affine_mul_reduce(
                        out=wdd_sb[:, g * FSH : (g + 1) * FSH],
                        accum_out=outd_sb[:, g : g + 1],
                        in0=wdd_sb[:, g * FSH : (g + 1) * FSH],
                        in1=zrep[:],
                        scale=1.0,
                        bias=0.0,
                    )
                for dt in range(DP // 512):
                    sl = slice(dt * 512, (dt + 1) * 512)
                    nc.scalar.copy(osbp[0:1, sl], pdp[0:1, sl])

            nc.sync.dma_start(out=outp_d.ap(), in_=osbp[:])
            nc.sync.dma_start(
                out=outd_d.ap()[:, 0 : NDVG // 2], in_=outd_sb[:, 0 : NDVG // 2]
            )
            nc.sync.dma_start(
                out=outd_d.ap()[:, NDVG // 2 : NDVG],
                in_=outd_sb[:, NDVG // 2 : NDVG],
            )

    nc.compile()
    return nc


def _get_nc():
    if "nc" not in _CACHE:
        _CACHE["nc"] = _build_nc()
    return _CACHE["nc"]


def make_in_maps(x, Wup, Wgatet, Wdownt, threshold):
    """Shard full inputs into the 8 per-core input maps (bf16 weights)."""
    x_flat = np.asarray(x, dtype=np.float32).reshape(D)
    xcol = np.ascontiguousarray(x_flat.reshape(NCD, 128).T).astype(BF)
    xf = x_flat.astype(BF)
    thr = np.asarray(threshold, dtype=np.float32).reshape(1)
    Wup = np.asarray(Wup, dtype=np.float32)
    Wgatet = np.asarray(Wgatet, dtype=np.float32)
    Wdownt = np.asarray(Wdownt, dtype=np.float32)
    in_maps = []
    for i in range(NCORES):
        sl = slice(i * FSH, (i + 1) * FSH)
        wg_slice = Wgatet[:, sl]                  # [D, FSH] d-major
        wu_slice = Wup[sl, :]                     # [FSH, D] f-major
        wd_slice = Wdownt[sl, :]                  # [FSH, D] f-major

        wgT = np.ascontiguousarray(wg_slice.T)    # [FSH, D] f-major
        wgd = (
            wgT[:FD].reshape(FDC, 128, D).transpose(1, 0, 2).reshape(128, FDC * D)
        )
        wud = (
            wu_slice[:FD]
            .reshape(FDC, 128, D)
            .transpose(1, 0, 2)
            .reshape(128, FDC * D)
        )
        wgp = (
            wg_slice[:, FD:]
            .reshape(NCD, 128, FP)
            .transpose(1, 0, 2)
            .reshape(128, NCD * FP)
        )
        wuT = np.ascontiguousarray(wu_slice.T)    # [D, FSH] d-major
        wup = (
            wuT[:, FD:]
            .reshape(NCD, 128, FP)
            .transpose(1, 0, 2)
            .reshape(128, NCD * FP)
        )
        wd_pad = np.zeros((NCF * 128, DP), dtype=np.float32)
        wd_pad[:FSH] = wd_slice[:, :DP]
        wdp = (
            wd_pad.reshape(NCF, 128, DP).transpose(1, 0, 2).reshape(128, NCF * DP)
        )
        wdT = np.ascontiguousarray(wd_slice.T)    # [D, FSH] d-major
        wdd = (
            wdT[DP:]
            .reshape(NDVG, 128, FSH)
            .transpose(1, 0, 2)
            .reshape(128, NDVG * FSH)
        )
        in_maps.append(
            {
                "xc": xcol,
                "xf": xf,
                "wgd": np.ascontiguousarray(wgd).astype(BF),
                "wud": np.ascontiguousarray(wud).astype(BF),
                "wgp": np.ascontiguousarray(wgp).astype(BF),
                "wup": np.ascontiguousarray(wup).astype(BF),
                "wdp": np.ascontiguousarray(wdp).astype(BF),
                "wdd": np.ascontiguousarray(wdd).astype(BF),
                "thr": thr,
            }
        )
    return in_maps


def run_sharded(x, Wup, Wgatet, Wdownt, threshold, trace=False, tmpdir=None):
    """Run on the 8 NeuronCores; returns (full_output, BassKernelResults)."""
    nc = _get_nc()
    in_maps = make_in_maps(x, Wup, Wgatet, Wdownt, threshold)
    res = run_bass_kernel_spmd(
        nc, in_maps, list(range(NCORES)), trace=trace, tmpdir=tmpdir
    )
    acc = np.zeros(D, dtype=np.float64)
    for r in res.results:
        acc[:DP] += r["outp"].reshape(DP).astype(np.float64)
        acc[DP:] += r["outd"].T.reshape(D - DP).astype(np.float64)
    out = acc.astype(np.float32).reshape(1, 1, D)
    return out, res


def kernel(x, Wup, Wgatet, Wdownt, threshold):
    out, _ = run_sharded(x, Wup, Wgatet, Wdownt, threshold)
    return out
